# revision 4
# baseline (speedup 1.0000x reference)
"""2D Haar DWT (single level) on Trainium2, 8 NeuronCores, pure data parallel.

Math: with Haar filters + symmetric pad + odd-phase downsample, the DWT
reduces to per-2x2-block butterflies over the input image x:
  ll = 0.5*(x00 + x01 + x10 + x11)   (top-left quadrant of output)
  lh = 0.5*(x00 + x01 - x10 - x11)   (bottom-left)
  hl = 0.5*(x00 - x01 + x10 - x11)   (top-right)
  hh = 0.5*(x00 - x01 - x10 + x11)   (bottom-right)

Per-core layout (8 images of 512x512 f32 per core):
  X[p, (B,w)]  f32: partition p holds image rows {B*128+p, B=0..3}, so
               height-pair rows (2q, 2q+1) sit on adjacent PARTITIONS.
  width pass   (DVE): T[p,(B,u,j)] bf16; u=0 col-pair sums, u=1 diffs.
               Two strided-read tensor_tensor ops per image: 1x mode at
               OUTPUT rate beats tensor_reduce (always 1x at INPUT rate)
               and beats GpSimd by ~3.5x.
  height pass  (PE): stationary 128x128 butterfly Bfly[p,po] with
               entries +-0.5 (scale fused, exact in bf16);
               out[po=(h,q), (u,j)] = 0.5*(T[2q] +- T[2q+1]) lands in
               PSUM as f32 -- no separate scale/cast pass anywhere.
  evacuate     PSUM->SBUF f32 copies split between ACT and DVE.
  out-DMA      (ACT HWDGE ring; in-DMAs ride the SP ring so the two
               streams overlap): one DMA per image; partition po=(h,q),
               free (B,w) maps to full DRAM rows [ll|hl] / [lh|hh] --
               2KB contiguous runs on both sides.

First image in-DMA/width-pass and first/last image out-DMAs are split in
halves to start the out stream early and shorten the exposed tail.
"""

import numpy as np
import ml_dtypes

import concourse.mybir as mybir
from concourse import bacc, tile
from concourse.bass_utils import run_bass_kernel_spmd

N_CORES = 8
BATCH = 64
B_PER = BATCH // N_CORES  # 8 images per core
H = W = 512

_nc_cache = None


def _butterfly_np():
    """Bfly[p, po] such that out[po,:] = sum_p Bfly[p,po] * T[p,:]:
    po = h*64+q; h=0: 0.5*(row 2q + row 2q+1); h=1: 0.5*(row 2q - row 2q+1)."""
    B = np.zeros((128, 128), np.float32)
    for q in range(64):
        B[2 * q, q] = 0.5
        B[2 * q + 1, q] = 0.5
        B[2 * q, 64 + q] = 0.5
        B[2 * q + 1, 64 + q] = -0.5
    return B.astype(ml_dtypes.bfloat16)


def build_bass():
    f32 = mybir.dt.float32
    bf16 = mybir.dt.bfloat16
    nc = bacc.Bacc(
        "TRN2", target_bir_lowering=False, debug=False, num_devices=N_CORES
    )
    inp = nc.dram_tensor("inputs", [B_PER, H, W], f32, kind="ExternalInput").ap()
    out = nc.dram_tensor("out", [B_PER, H, W], f32, kind="ExternalOutput").ap()
    bfly_dram = nc.inline_tensor(_butterfly_np(), name="bfly")

    with tile.TileContext(nc) as tc:
        with tc.tile_pool(name="p", bufs=3) as pool, tc.tile_pool(
            name="ps", bufs=8, space="PSUM"
        ) as ppool:
            Bsb = pool.tile([128, 128], bf16, tag="bfly", bufs=1)
            # ACT ring is idle at startup; keep the SP ring clear for the
            # first input image.
            nc.scalar.dma_start(out=Bsb[:], in_=bfly_dram.ap())

            for i in range(B_PER):
                X = pool.tile([128, 2048], f32, tag="X", bufs=4)
                src = inp[i].rearrange("(B p) w -> p B w", p=128)
                if i == 0:  # halve the first fill to start compute sooner
                    nc.sync.dma_start(out=X[:, :1024], in_=src[:, :2])
                    nc.sync.dma_start(out=X[:, 1024:], in_=src[:, 2:])
                else:
                    nc.sync.dma_start(out=X[:], in_=src)

                T = pool.tile([128, 2048], bf16, tag="T", bufs=3)
                Xv = X[:].rearrange("p (B j two) -> p B j two", B=4, two=2)
                Tv = T[:].rearrange("p (B u j) -> p B u j", B=4, u=2)
                with nc.allow_low_precision(reason="bf16 DWT intermediates"):
                    spans = [(0, 2), (2, 4)] if i == 0 else [(0, 4)]
                    for lo, hi in spans:
                        nc.vector.tensor_add(
                            out=Tv[:, lo:hi, 0],
                            in0=Xv[:, lo:hi, :, 0],
                            in1=Xv[:, lo:hi, :, 1],
                        )
                        nc.vector.tensor_sub(
                            out=Tv[:, lo:hi, 1],
                            in0=Xv[:, lo:hi, :, 0],
                            in1=Xv[:, lo:hi, :, 1],
                        )

                Y = pool.tile([128, 2048], f32, tag="Y", bufs=3)
                for b in range(4):
                    P = ppool.tile([128, 512], f32, tag="P")
                    nc.tensor.matmul(
                        P[:], Bsb[:], T[:, b * 512 : (b + 1) * 512],
                        start=True, stop=True,
                    )
                    dst = Y[:, b * 512 : (b + 1) * 512]
                    if b % 2 == 0:
                        nc.scalar.copy(out=dst, in_=P[:])
                    else:
                        nc.vector.tensor_copy(out=dst, in_=P[:])

                # DMA APs max out at 3 dims, and partitions po=(h,q) interleave
                # with B in the DRAM row index -- so emit one DMA per output
                # half h (64 partitions, [q, B, w] on the DRAM side).
                bspans = [(0, 2), (2, 4)] if i in (0, B_PER - 1) else [(0, 4)]
                for blo, bhi in bspans:  # early first out / short tail
                    for h in range(2):
                        dst = out[i][h * 256 : (h + 1) * 256].rearrange(
                            "(B q) w -> q B w", B=4
                        )[:, blo:bhi]
                        srcY = Y[h * 64 : (h + 1) * 64].rearrange(
                            "p (B w) -> p B w", B=4
                        )[:, blo:bhi]
                        nc.scalar.dma_start(out=dst, in_=srcY)

    nc.compile()
    return nc


def kernel(**inputs):
    global _nc_cache
    x = np.ascontiguousarray(
        np.asarray(inputs["inputs"], dtype=np.float32).reshape(BATCH, H, W)
    )
    if _nc_cache is None:
        _nc_cache = build_bass()
    nc = _nc_cache
    in_maps = [
        {"inputs": x[i * B_PER : (i + 1) * B_PER]} for i in range(N_CORES)
    ]
    res = run_bass_kernel_spmd(nc, in_maps, core_ids=list(range(N_CORES))).results
    out = np.concatenate([res[i]["out"] for i in range(N_CORES)], axis=0)
    return out.reshape(BATCH, H, W, 1)
